# revision 36
# baseline (speedup 1.0000x reference)
"""Trainium2 Bass kernel for nn_BootstrappedCE (topk_masking).

Computes: BCE loss over 16x1x1024x1024 probabilities/targets, then the mean
of the top 25% loss values (k = N/4), returning (mean, 0.25) — matching the
reference's post-warmup branch. For it < 1000 it returns (mean of all losses,
1.0).

Strategy (data-parallel over batch, 8 cores, 2 images each):
  The top-k mean is computed via the exact CVaR identity
      mean_topk = tau + sum(relu(loss - tau)) / k
  which holds exactly when tau is the k-th largest loss, and is SECOND-ORDER
  insensitive to tau error (d/dtau = (1 - C(tau)/k) -> 0 at the true
  quantile). A cheap host-side pilot (stride-64 subsample, ~260k elements)
  estimates tau to ~1e-3, giving ~1e-9 final error. Each core then does ONE
  memory-bound pass over its shard computing sum(relu(loss - tau)) and
  count(loss > tau) using ScalarE's fused activation+accumulate; the count is
  only a guard — if the pilot tau were somehow off (|C - k| > 2% k), we
  bisect with exact device counts and rerun (never triggers on rand data).

  Per [128, 2048] chunk: ACT: lp=ln(p), lq=ln(1-p), relu(loss-tau)+accum;
  DVE: g=lq-lp, f=t*g, loss=f-lq (TT ops), count via tensor_scalar is_gt
  +accum. Selection ops for chunk i are emitted after chunk i+1's ln ops so
  the cross-engine dependency chain pipelines instead of serializing.
"""

import numpy as np

import concourse.bass as bass  # noqa: F401  (bass types used via tile/bacc)
import concourse.mybir as mybir
import concourse.tile as tile
from concourse import bacc
from concourse.bass_utils import run_bass_kernel_spmd

# Problem shape (hardcoded per contract; kernel.py must be self-contained).
B, H, W = 16, 1024, 1024
N_TOTAL = B * H * W
NCORES = 8
PER_CORE = N_TOTAL // NCORES          # 2_097_152
P = 128                               # SBUF partitions
FREE = PER_CORE // P                  # 16384
# Ragged chunking: small first chunks cut the pipeline-fill bubble (first
# compute waits only on a small DMA); small last chunks cut the serial
# drain chain. Sizes must sum to FREE.
CHUNKS = [2048] * 8
NCH = len(CHUNKS)
# Boundary chunks are processed in sub-pieces within their (uniform-size)
# tiles: the first piece of chunk 0 computes while the rest of the tile is
# still DMAing (cuts pipeline fill), and the last chunk's serial drain
# chain shrinks to its final small piece. Pieces are (start, size) within
# the chunk. All tiles stay [P, 2048] so the tile pools see one shape.
PIECES_FIRST = [(0, 512), (512, 1536)]
PIECES_LAST = [(0, 1536), (1536, 512)]
SEL_FORCE_ACT = False  # True: all selection on ACT (frees DVE)
CAST_T = True          # True: t loaded as f16 via gpsimd cast-DMA


def _make_plan():
    plan = []
    for i in range(NCH):
        ps = (PIECES_FIRST if i == 0
              else (PIECES_LAST if i == NCH - 1 else [(0, CHUNKS[i])]))
        for s, n in ps:
            plan.append((i, s, n, (i % 2 == 1) and not SEL_FORCE_ACT))
    return plan


SEL_COLS = [(d, n) for (_, _, n, d) in _make_plan()]
NCOLS = len(SEL_COLS)

START_WARM = 1000
TOP_P = 0.25

# Toggles for perf experiments (test.py flips these; defaults = best known).
G_ON_GPSIMD = False   # the lq-lp subtract on GPSIMD instead of DVE
COUNT_ON = False      # emit the count guard op at all
PREC = "f16"          # "f32" | "f16": dtype of log/loss intermediates
TTR_SEL = False       # TTR fp16 crashes the exec unit on HW; keep False
TRACE = False         # test.py sets True to get exec_time_ns
LAST_RESULTS = None   # BassKernelResults of the last run (for test.py)

_CACHED_NC = None


def _build_nc():
    global SEL_COLS, NCOLS
    plan = _make_plan()
    SEL_COLS = [(d, n) for (_, _, n, d) in plan]
    NCOLS = len(SEL_COLS)
    nc = bacc.Bacc("TRN2", target_bir_lowering=False, debug=False,
                   enable_asserts=False, num_devices=NCORES)
    p_in = nc.dram_tensor("p_in", [P, FREE], mybir.dt.float32, kind="ExternalInput")
    t_in = nc.dram_tensor("t_in", [P, FREE], mybir.dt.float32, kind="ExternalInput")
    tau_in = nc.dram_tensor("tau_in", [P, 1], mybir.dt.float32, kind="ExternalInput")
    ntau_in = nc.dram_tensor("ntau_in", [P, 1], mybir.dt.float32, kind="ExternalInput")
    out_ra = nc.dram_tensor("out_ra", [P, NCOLS], mybir.dt.float32, kind="ExternalOutput")
    out_cnt = nc.dram_tensor("out_cnt", [P, NCOLS], mybir.dt.float32, kind="ExternalOutput")

    f32 = mybir.dt.float32
    AF = mybir.ActivationFunctionType
    OP = mybir.AluOpType
    g_eng = None  # set inside

    maxc = max(CHUNKS)
    with tile.TileContext(nc) as tc:
        with tc.tile_pool(name="io", bufs=4) as io_pool, \
             tc.tile_pool(name="work", bufs=3) as work, \
             tc.tile_pool(name="junkp", bufs=2) as junkp, \
             tc.tile_pool(name="accs", bufs=1) as accs:
            g_eng = nc.gpsimd if G_ON_GPSIMD else nc.vector
            wdt = f32 if PREC == "f32" else mybir.dt.float16
            do_cast = PREC != "f32" and CAST_T
            t_dt = wdt if do_cast else f32
            cast_eng = nc.gpsimd if do_cast else nc.sync
            sel_flag = {(i, s): d for (i, s, _, d) in plan}

            def pieces_for(i):
                if i == 0:
                    return PIECES_FIRST
                if i == NCH - 1:
                    return PIECES_LAST
                return [(0, CHUNKS[i])]

            # First p piece DMA goes out before anything else so the first
            # LN starts as early as possible.
            ch0 = CHUNKS[0]
            pt0 = io_pool.tile([P, ch0], f32, tag="p")
            tt0 = io_pool.tile([P, ch0], t_dt, tag="t")
            for s, n in pieces_for(0):
                nc.sync.dma_start(pt0[:, s:s + n], p_in.ap()[:, s:s + n])
                cast_eng.dma_start(tt0[:, s:s + n], t_in.ap()[:, s:s + n])

            tau = accs.tile([P, 1], f32)
            ntau = accs.tile([P, 1], f32)
            racc = accs.tile([P, NCOLS], f32)
            cacc = accs.tile([P, NCOLS], f32) if COUNT_ON else None
            nc.sync.dma_start(tau[:], tau_in.ap())
            nc.sync.dma_start(ntau[:], ntau_in.ap())

            pending = []  # (loss_ap, col, on_dve) awaiting selection ops

            def emit_selection(loss_ap, col, on_dve, n):
                if COUNT_ON:
                    junk1 = junkp.tile([P, 2048], wdt, tag="junk1")
                    nc.vector.tensor_scalar(
                        out=junk1[:, :n], in0=loss_ap, scalar1=tau[:],
                        scalar2=None, op0=OP.is_gt, op1=OP.add,
                        accum_out=cacc[:, col:col + 1])
                junk2 = junkp.tile([P, 2048], wdt, tag="junk2")
                if on_dve:
                    # accum = sum(max(loss, tau)); host subtracts n*tau.
                    nc.vector.tensor_scalar(
                        out=junk2[:, :n], in0=loss_ap, scalar1=tau[:],
                        scalar2=None, op0=OP.max, op1=OP.add,
                        accum_out=racc[:, col:col + 1])
                else:
                    # accum = sum(relu(loss - tau))
                    nc.scalar.activation(junk2[:, :n], loss_ap, AF.Relu,
                                         bias=ntau[:], scale=1.0,
                                         accum_out=racc[:, col:col + 1])

            col = 0
            off = 0
            for i, ch in enumerate(CHUNKS):
                base = off
                off += ch
                if i == 0:
                    pt, tt = pt0, tt0
                else:
                    pt = io_pool.tile([P, ch], f32, tag="p")
                    tt = io_pool.tile([P, ch], t_dt, tag="t")
                    for s, n in pieces_for(i):
                        nc.sync.dma_start(pt[:, s:s + n],
                                          p_in.ap()[:, base + s:base + s + n])
                        # HWDGE can't cast; gpsimd DGE casts f32->f16.
                        cast_eng.dma_start(tt[:, s:s + n],
                                           t_in.ap()[:, base + s:base + s + n])

                lp = work.tile([P, ch], wdt, tag="lp")
                if PREC != "f32":
                    lq = work.tile([P, ch], wdt, tag="lq")
                else:
                    lq = pt
                for s, n in pieces_for(i):
                    sl = slice(s, s + n)
                    nc.scalar.activation(lp[:, sl], pt[:, sl], AF.Ln)
                    nc.scalar.activation(lq[:, sl], pt[:, sl], AF.Ln,
                                         bias=1.0, scale=-1.0)       # ln(1-p)
                    # g = lq - lp  (onto lp)
                    g_eng.tensor_tensor(out=lp[:, sl], in0=lq[:, sl],
                                        in1=lp[:, sl], op=OP.subtract)
                    # f = t * g  (onto tt)
                    nc.vector.tensor_tensor(out=tt[:, sl], in0=tt[:, sl],
                                            in1=lp[:, sl], op=OP.mult)
                    # loss = f - lq  (onto lq)
                    nc.vector.tensor_tensor(out=lq[:, sl], in0=tt[:, sl],
                                            in1=lq[:, sl], op=OP.subtract)
                    pending.append((lq[:, sl], col, sel_flag[(i, s)], n))
                    col += 1
                    # Selection ops lag one piece so engines pipeline.
                    if len(pending) > 1:
                        emit_selection(*pending.pop(0))
            while pending:
                emit_selection(*pending.pop(0))

            nc.sync.dma_start(out_ra.ap(), racc[:])
            if COUNT_ON:
                nc.sync.dma_start(out_cnt.ap(), cacc[:])
    nc.compile()
    return nc


def _get_nc():
    global _CACHED_NC
    if _CACHED_NC is None:
        _CACHED_NC = _build_nc()
    return _CACHED_NC


def _pilot(p_flat, t_flat, k):
    """Host pilot on a strided subsample: estimate the k-th largest loss tau
    and the expected A = sum(relu(loss - tau)) for the sanity guard."""
    ps = p_flat[::64].astype(np.float64)
    ts = t_flat[::64].astype(np.float64)
    loss = -(ts * np.clip(np.log(ps), -100.0, None)
             + (1.0 - ts) * np.clip(np.log1p(-ps), -100.0, None))
    n = loss.size
    if k <= 0:
        tau = 0.0
    else:
        kk = min(n - 1, max(1, int(round(n * (k / N_TOTAL)))))
        tau = float(np.partition(loss, n - kk)[n - kk])
    a_pred = float(np.maximum(loss - tau, 0.0).mean()) * N_TOTAL
    return tau, a_pred


def _run_device_pass(nc, p_full, t_full, tau):
    """One full pass: returns (A = sum(relu(loss - tau)), C = count(loss > tau))."""
    global LAST_RESULTS
    in_maps = []
    tau_arr = np.full((P, 1), tau, np.float32)
    ntau_arr = np.full((P, 1), -tau, np.float32)
    per_img = PER_CORE // (B // NCORES)  # elements per image
    imgs_per_core = B // NCORES
    for c in range(NCORES):
        lo = c * imgs_per_core * per_img
        hi = lo + PER_CORE
        in_maps.append({
            "p_in": p_full[lo:hi].reshape(P, FREE),
            "t_in": t_full[lo:hi].reshape(P, FREE),
            "tau_in": tau_arr,
            "ntau_in": ntau_arr,
        })
    res = run_bass_kernel_spmd(nc, in_maps, core_ids=list(range(NCORES)),
                               trace=TRACE)
    LAST_RESULTS = res
    A = 0.0
    C = 0.0
    n_max_elems = 0
    for c in range(NCORES):
        ra = res.results[c]["out_ra"].astype(np.float64)
        for i, (on_dve, nel) in enumerate(SEL_COLS):
            A += float(ra[:, i].sum())
            if on_dve:
                n_max_elems += P * nel
        if COUNT_ON:
            C += float(res.results[c]["out_cnt"].astype(np.float64).sum())
    A -= tau * n_max_elems   # max(loss,tau) columns carry a +tau per element
    return A, C


def kernel(input, target, it):
    p_full = np.ascontiguousarray(np.asarray(input, dtype=np.float32)).ravel()
    t_full = np.ascontiguousarray(np.asarray(target, dtype=np.float32)).ravel()
    it_val = int(np.asarray(it))
    nc = _get_nc()

    if it_val < START_WARM:
        # Plain mean of all losses: tau=0 makes relu(loss-0)=loss (loss >= 0).
        _, a_pred = _pilot(p_full, t_full, 0)
        A, _ = _run_device_pass(nc, p_full, t_full, 0.0)
        assert abs(A - a_pred) <= 0.2 * abs(a_pred) + 1e-6, (A, a_pred)
        return np.float32(A / N_TOTAL), 1.0

    k = int(N_TOTAL * TOP_P)
    tau, a_pred = _pilot(p_full, t_full, k)
    A, C = _run_device_pass(nc, p_full, t_full, tau)
    # Guard: the device A must agree with the pilot's prediction to ~20%
    # (iid sampling errors are ~0.3%; a gross mismatch means the strided
    # pilot was unrepresentative). Fall back to exact bisection with the
    # count variant of the kernel in that case.
    if abs(A - a_pred) > 0.2 * abs(a_pred) + 1e-6:
        global COUNT_ON, _CACHED_NC
        COUNT_ON, _CACHED_NC = True, None
        nc = _get_nc()
        A, C = _run_device_pass(nc, p_full, t_full, tau)
        lo_t, hi_t = 0.0, 101.0
        for _ in range(40):
            if abs(C - k) <= 0.02 * k:
                break
            if C > k:
                lo_t = tau
            else:
                hi_t = tau
            tau = 0.5 * (lo_t + hi_t)
            A, C = _run_device_pass(nc, p_full, t_full, tau)
    return np.float32(tau + A / k), TOP_P


# revision 40
# speedup vs baseline: 1.0230x; 1.0230x over previous
"""Trainium2 Bass kernel for nn_BootstrappedCE (topk_masking).

Computes: BCE loss over 16x1x1024x1024 probabilities/targets, then the mean
of the top 25% loss values (k = N/4), returning (mean, 0.25) — matching the
reference's post-warmup branch. For it < 1000 it returns (mean of all losses,
1.0).

Strategy (data-parallel over batch, 8 cores, 2 images each):
  The top-k mean is computed via the exact CVaR identity
      mean_topk = tau + sum(relu(loss - tau)) / k
  which holds exactly when tau is the k-th largest loss, and is SECOND-ORDER
  insensitive to tau error (d/dtau = (1 - C(tau)/k) -> 0 at the true
  quantile). A cheap host-side pilot (stride-64 subsample, ~260k elements)
  estimates tau to ~1e-3, giving ~1e-9 final error from the identity. Each
  core then does ONE memory-bound pass over its shard accumulating
  sum(relu(loss - tau)); the host combines the per-lane partials in f64.
  Guard: the pilot also predicts A = sum(relu(loss - tau)); if the device
  value disagrees grossly (unrepresentative strided sample — impossible for
  iid data), we fall back to a count-instrumented kernel and bisect tau
  against exact device counts.

  Per-core pass, per [128, 2048] chunk of the [128, 16384] shard:
  DMA: p as f32 (HWDGE), t cast f32->f16 (gpsimd DGE); ACT: lp=ln(p),
  lq=ln(1-p) (scale=-1, bias=1) written as f16; DVE (f16 tensor_tensor runs
  in 2x mode): g=lq-lp, f=t*g, loss=f-lq. Selection alternates per chunk to
  balance engines: ACT relu(loss-tau)+accum_out, or DVE
  tensor_scalar(max,tau)+accum (host subtracts n*tau). Selection ops lag one
  chunk so the ACT->DVE->ACT chain pipelines; boundary chunks are processed
  in sub-pieces to shorten pipeline fill and drain. Accuracy vs the f32
  reference: ~2e-5 relative (DVE computes f32 internally; only lp/lq/loss
  storage rounds to f16).
"""

import numpy as np

import concourse.mybir as mybir
import concourse.tile as tile
from concourse import bacc
from concourse.bass_utils import run_bass_kernel_spmd

# Problem shape (hardcoded per contract; kernel.py must be self-contained).
B, H, W = 16, 1024, 1024
N_TOTAL = B * H * W
NCORES = 8
PER_CORE = N_TOTAL // NCORES          # 2_097_152
P = 128                               # SBUF partitions
FREE = PER_CORE // P                  # 16384
# Ragged chunking: small first chunks cut the pipeline-fill bubble (first
# compute waits only on a small DMA); small last chunks cut the serial
# drain chain. Sizes must sum to FREE.
CHUNKS = [2048] * 8
NCH = len(CHUNKS)
# Boundary chunks are processed in sub-pieces within their (uniform-size)
# tiles: the first piece of chunk 0 computes while the rest of the tile is
# still DMAing (cuts pipeline fill), and the last chunk's serial drain
# chain shrinks to its final small piece. Pieces are (start, size) within
# the chunk. All tiles stay [P, 2048] so the tile pools see one shape.
PIECES_FIRST = [(0, 512), (512, 1536)]
PIECES_LAST = [(0, 1536), (1536, 512)]
SEL_FORCE_ACT = False  # True: all selection on ACT (frees DVE)
CAST_T = True          # True: t loaded as f16 via gpsimd cast-DMA


def _make_plan():
    plan = []
    for i in range(NCH):
        ps = (PIECES_FIRST if i == 0
              else (PIECES_LAST if i == NCH - 1 else [(0, CHUNKS[i])]))
        for s, n in ps:
            plan.append((i, s, n, (i % 2 == 1) and not SEL_FORCE_ACT))
    return plan


SEL_COLS = [(d, n) for (_, _, n, d) in _make_plan()]
NCOLS = len(SEL_COLS)

START_WARM = 1000
TOP_P = 0.25

# Toggles for perf experiments (test.py flips these; defaults = best known).
G_ON_GPSIMD = False   # the lq-lp subtract on GPSIMD instead of DVE
COUNT_ON = False      # emit the count guard op at all
PREC = "f16"          # "f32" | "f16": dtype of log/loss intermediates
TRACE = False         # test.py sets True to get exec_time_ns
LAST_RESULTS = None   # BassKernelResults of the last run (for test.py)

_CACHED_NC = None


def _build_nc():
    global SEL_COLS, NCOLS
    plan = _make_plan()
    SEL_COLS = [(d, n) for (_, _, n, d) in plan]
    NCOLS = len(SEL_COLS)
    nc = bacc.Bacc("TRN2", target_bir_lowering=False, debug=False,
                   enable_asserts=False, num_devices=NCORES)
    p_in = nc.dram_tensor("p_in", [P, FREE], mybir.dt.float32, kind="ExternalInput")
    t_in = nc.dram_tensor("t_in", [P, FREE], mybir.dt.float32, kind="ExternalInput")
    tau_in = nc.dram_tensor("tau_in", [P, 1], mybir.dt.float32, kind="ExternalInput")
    ntau_in = nc.dram_tensor("ntau_in", [P, 1], mybir.dt.float32, kind="ExternalInput")
    out_ra = nc.dram_tensor("out_ra", [P, NCOLS], mybir.dt.float32, kind="ExternalOutput")
    out_cnt = nc.dram_tensor("out_cnt", [P, NCOLS], mybir.dt.float32, kind="ExternalOutput")

    f32 = mybir.dt.float32
    AF = mybir.ActivationFunctionType
    OP = mybir.AluOpType

    with tile.TileContext(nc) as tc:
        with tc.tile_pool(name="io", bufs=4) as io_pool, \
             tc.tile_pool(name="work", bufs=3) as work, \
             tc.tile_pool(name="junkp", bufs=2) as junkp, \
             tc.tile_pool(name="accs", bufs=1) as accs:
            g_eng = nc.gpsimd if G_ON_GPSIMD else nc.vector
            wdt = f32 if PREC == "f32" else mybir.dt.float16
            do_cast = PREC != "f32" and CAST_T
            t_dt = wdt if do_cast else f32
            cast_eng = nc.gpsimd if do_cast else nc.sync
            sel_flag = {(i, s): d for (i, s, _, d) in plan}

            def pieces_for(i):
                if i == 0:
                    return PIECES_FIRST
                if i == NCH - 1:
                    return PIECES_LAST
                return [(0, CHUNKS[i])]

            # First p piece DMA goes out before anything else so the first
            # LN starts as early as possible.
            ch0 = CHUNKS[0]
            pt0 = io_pool.tile([P, ch0], f32, tag="p")
            tt0 = io_pool.tile([P, ch0], t_dt, tag="t")
            for s, n in pieces_for(0):
                nc.sync.dma_start(pt0[:, s:s + n], p_in.ap()[:, s:s + n])
                cast_eng.dma_start(tt0[:, s:s + n], t_in.ap()[:, s:s + n])

            tau = accs.tile([P, 1], f32)
            ntau = accs.tile([P, 1], f32)
            racc = accs.tile([P, NCOLS], f32)
            cacc = accs.tile([P, NCOLS], f32) if COUNT_ON else None
            nc.sync.dma_start(tau[:], tau_in.ap())
            nc.sync.dma_start(ntau[:], ntau_in.ap())

            pending = []  # (loss_ap, col, on_dve) awaiting selection ops

            def emit_selection(loss_ap, col, on_dve, n):
                if COUNT_ON:
                    junk1 = junkp.tile([P, 2048], wdt, tag="junk1")
                    nc.vector.tensor_scalar(
                        out=junk1[:, :n], in0=loss_ap, scalar1=tau[:],
                        scalar2=None, op0=OP.is_gt, op1=OP.add,
                        accum_out=cacc[:, col:col + 1])
                junk2 = junkp.tile([P, 2048], wdt, tag="junk2")
                if on_dve:
                    # accum = sum(max(loss, tau)); host subtracts n*tau.
                    nc.vector.tensor_scalar(
                        out=junk2[:, :n], in0=loss_ap, scalar1=tau[:],
                        scalar2=None, op0=OP.max, op1=OP.add,
                        accum_out=racc[:, col:col + 1])
                else:
                    # accum = sum(relu(loss - tau))
                    nc.scalar.activation(junk2[:, :n], loss_ap, AF.Relu,
                                         bias=ntau[:], scale=1.0,
                                         accum_out=racc[:, col:col + 1])

            col = 0
            off = 0
            for i, ch in enumerate(CHUNKS):
                base = off
                off += ch
                if i == 0:
                    pt, tt = pt0, tt0
                else:
                    pt = io_pool.tile([P, ch], f32, tag="p")
                    tt = io_pool.tile([P, ch], t_dt, tag="t")
                    for s, n in pieces_for(i):
                        nc.sync.dma_start(pt[:, s:s + n],
                                          p_in.ap()[:, base + s:base + s + n])
                        # HWDGE can't cast; gpsimd DGE casts f32->f16.
                        cast_eng.dma_start(tt[:, s:s + n],
                                           t_in.ap()[:, base + s:base + s + n])

                lp = work.tile([P, ch], wdt, tag="lp")
                if PREC != "f32":
                    lq = work.tile([P, ch], wdt, tag="lq")
                else:
                    lq = pt
                for s, n in pieces_for(i):
                    sl = slice(s, s + n)
                    nc.scalar.activation(lp[:, sl], pt[:, sl], AF.Ln)
                    nc.scalar.activation(lq[:, sl], pt[:, sl], AF.Ln,
                                         bias=1.0, scale=-1.0)       # ln(1-p)
                    # g = lq - lp  (onto lp)
                    g_eng.tensor_tensor(out=lp[:, sl], in0=lq[:, sl],
                                        in1=lp[:, sl], op=OP.subtract)
                    # f = t * g  (onto tt)
                    nc.vector.tensor_tensor(out=tt[:, sl], in0=tt[:, sl],
                                            in1=lp[:, sl], op=OP.mult)
                    # loss = f - lq  (onto lq)
                    nc.vector.tensor_tensor(out=lq[:, sl], in0=tt[:, sl],
                                            in1=lq[:, sl], op=OP.subtract)
                    pending.append((lq[:, sl], col, sel_flag[(i, s)], n))
                    col += 1
                    # Selection ops lag one piece so engines pipeline.
                    if len(pending) > 1:
                        emit_selection(*pending.pop(0))
            while pending:
                emit_selection(*pending.pop(0))

            nc.sync.dma_start(out_ra.ap(), racc[:])
            if COUNT_ON:
                nc.sync.dma_start(out_cnt.ap(), cacc[:])
    nc.compile()
    return nc


def _get_nc():
    global _CACHED_NC
    if _CACHED_NC is None:
        _CACHED_NC = _build_nc()
    return _CACHED_NC


def _pilot(p_flat, t_flat, k):
    """Host pilot on a strided subsample: estimate the k-th largest loss tau
    and the expected A = sum(relu(loss - tau)) for the sanity guard."""
    ps = p_flat[::64].astype(np.float64)
    ts = t_flat[::64].astype(np.float64)
    loss = -(ts * np.clip(np.log(ps), -100.0, None)
             + (1.0 - ts) * np.clip(np.log1p(-ps), -100.0, None))
    n = loss.size
    if k <= 0:
        tau = 0.0
    else:
        kk = min(n - 1, max(1, int(round(n * (k / N_TOTAL)))))
        tau = float(np.partition(loss, n - kk)[n - kk])
    a_pred = float(np.maximum(loss - tau, 0.0).mean()) * N_TOTAL
    return tau, a_pred


def _run_device_pass(nc, p_full, t_full, tau):
    """One full pass: returns (A = sum(relu(loss - tau)), C = count(loss > tau))."""
    global LAST_RESULTS
    in_maps = []
    tau_arr = np.full((P, 1), tau, np.float32)
    ntau_arr = np.full((P, 1), -tau, np.float32)
    per_img = PER_CORE // (B // NCORES)  # elements per image
    imgs_per_core = B // NCORES
    for c in range(NCORES):
        lo = c * imgs_per_core * per_img
        hi = lo + PER_CORE
        in_maps.append({
            "p_in": p_full[lo:hi].reshape(P, FREE),
            "t_in": t_full[lo:hi].reshape(P, FREE),
            "tau_in": tau_arr,
            "ntau_in": ntau_arr,
        })
    res = run_bass_kernel_spmd(nc, in_maps, core_ids=list(range(NCORES)),
                               trace=TRACE)
    LAST_RESULTS = res
    A = 0.0
    C = 0.0
    n_max_elems = 0
    for c in range(NCORES):
        ra = res.results[c]["out_ra"].astype(np.float64)
        for i, (on_dve, nel) in enumerate(SEL_COLS):
            A += float(ra[:, i].sum())
            if on_dve:
                n_max_elems += P * nel
        if COUNT_ON:
            C += float(res.results[c]["out_cnt"].astype(np.float64).sum())
    A -= tau * n_max_elems   # max(loss,tau) columns carry a +tau per element
    return A, C


def kernel(input, target, it):
    p_full = np.ascontiguousarray(np.asarray(input, dtype=np.float32)).ravel()
    t_full = np.ascontiguousarray(np.asarray(target, dtype=np.float32)).ravel()
    it_val = int(np.asarray(it))
    nc = _get_nc()

    if it_val < START_WARM:
        # Plain mean of all losses: tau=0 makes relu(loss-0)=loss (loss >= 0).
        _, a_pred = _pilot(p_full, t_full, 0)
        A, _ = _run_device_pass(nc, p_full, t_full, 0.0)
        assert abs(A - a_pred) <= 0.2 * abs(a_pred) + 1e-6, (A, a_pred)
        return np.float32(A / N_TOTAL), 1.0

    k = int(N_TOTAL * TOP_P)
    tau, a_pred = _pilot(p_full, t_full, k)
    A, C = _run_device_pass(nc, p_full, t_full, tau)
    # Guard: the device A must agree with the pilot's prediction to ~20%
    # (iid sampling errors are ~0.3%; a gross mismatch means the strided
    # pilot was unrepresentative). Fall back to exact bisection with the
    # count variant of the kernel in that case.
    if abs(A - a_pred) > 0.2 * abs(a_pred) + 1e-6:
        global COUNT_ON, _CACHED_NC
        COUNT_ON, _CACHED_NC = True, None
        nc = _get_nc()
        A, C = _run_device_pass(nc, p_full, t_full, tau)
        lo_t, hi_t = 0.0, 101.0
        for _ in range(40):
            if abs(C - k) <= 0.02 * k:
                break
            if C > k:
                lo_t = tau
            else:
                hi_t = tau
            tau = 0.5 * (lo_t + hi_t)
            A, C = _run_device_pass(nc, p_full, t_full, tau)
    return np.float32(tau + A / k), TOP_P


# revision 45
# speedup vs baseline: 1.0299x; 1.0068x over previous
"""Trainium2 Bass kernel for nn_BootstrappedCE (topk_masking).

Computes: BCE loss over 16x1x1024x1024 probabilities/targets, then the mean
of the top 25% loss values (k = N/4), returning (mean, 0.25) — matching the
reference's post-warmup branch. For it < 1000 it returns (mean of all losses,
1.0).

Strategy (data-parallel over batch, 8 cores, 2 images each):
  The top-k mean is computed via the exact CVaR identity
      mean_topk = tau + sum(relu(loss - tau)) / k
  which holds exactly when tau is the k-th largest loss, and is SECOND-ORDER
  insensitive to tau error (d/dtau = (1 - C(tau)/k) -> 0 at the true
  quantile). A cheap host-side pilot (stride-64 subsample, ~260k elements)
  estimates tau to ~1e-3, giving ~1e-9 final error from the identity. Each
  core then does ONE memory-bound pass over its shard accumulating
  sum(relu(loss - tau)); the host combines the per-lane partials in f64.
  Guard: the pilot also predicts A = sum(relu(loss - tau)); if the device
  value disagrees grossly (unrepresentative strided sample — impossible for
  iid data), we fall back to a count-instrumented kernel and bisect tau
  against exact device counts.

  Per-core pass, per [128, 2048] chunk of the [128, 16384] shard:
  DMA: p as f32 (HWDGE), t cast f32->f16 (gpsimd DGE); ACT: lp=ln(p),
  lq=ln(1-p) (scale=-1, bias=1) written as f16; DVE (f16 tensor_tensor runs
  in 2x mode): g=lq-lp, f=t*g, loss=f-lq. Selection alternates per chunk to
  balance engines: ACT relu(loss-tau)+accum_out, or DVE
  tensor_scalar(max,tau)+accum (host subtracts n*tau). Selection ops lag one
  chunk so the ACT->DVE->ACT chain pipelines; boundary chunks are processed
  in sub-pieces to shorten pipeline fill and drain. Accuracy vs the f32
  reference: ~2e-5 relative (DVE computes f32 internally; only lp/lq/loss
  storage rounds to f16).
"""

import numpy as np

import concourse.mybir as mybir
import concourse.tile as tile
from concourse import bacc
from concourse.bass_utils import run_bass_kernel_spmd

# Problem shape (hardcoded per contract; kernel.py must be self-contained).
B, H, W = 16, 1024, 1024
N_TOTAL = B * H * W
NCORES = 8
PER_CORE = N_TOTAL // NCORES          # 2_097_152
P = 128                               # SBUF partitions
FREE = PER_CORE // P                  # 16384
# Ragged chunking: small first chunks cut the pipeline-fill bubble (first
# compute waits only on a small DMA); small last chunks cut the serial
# drain chain. Sizes must sum to FREE.
CHUNKS = [2048] * 8
NCH = len(CHUNKS)
# Boundary chunks are processed in sub-pieces within their (uniform-size)
# tiles: the first piece of chunk 0 computes while the rest of the tile is
# still DMAing (cuts pipeline fill), and the last chunk's serial drain
# chain shrinks to its final small piece. Pieces are (start, size) within
# the chunk. All tiles stay [P, 2048] so the tile pools see one shape.
PIECES_FIRST = [(0, 512), (512, 1536)]
PIECES_LAST = [(0, 1536), (1536, 512)]
SEL_FORCE_ACT = False  # True: all selection on ACT (frees DVE)
CAST_T = True          # True: t loaded as f16 via gpsimd cast-DMA


def _make_plan():
    plan = []
    for i in range(NCH):
        ps = (PIECES_FIRST if i == 0
              else (PIECES_LAST if i == NCH - 1 else [(0, CHUNKS[i])]))
        for s, n in ps:
            plan.append((i, s, n, (i % 2 == 1) and not SEL_FORCE_ACT))
    return plan


SEL_COLS = [(d, n) for (_, _, n, d) in _make_plan()]
NCOLS = len(SEL_COLS)

START_WARM = 1000
TOP_P = 0.25

# Toggles for perf experiments (test.py flips these; defaults = best known).
G_ON_GPSIMD = False   # the lq-lp subtract on GPSIMD instead of DVE
COUNT_ON = False      # emit the count guard op at all
PREC = "f16"          # "f32" | "f16": dtype of log/loss intermediates
TRACE = False         # test.py sets True to get exec_time_ns
LAST_RESULTS = None   # BassKernelResults of the last run (for test.py)

_CACHED_NC = None


def _build_nc():
    global SEL_COLS, NCOLS
    plan = _make_plan()
    SEL_COLS = [(d, n) for (_, _, n, d) in plan]
    NCOLS = len(SEL_COLS)
    nc = bacc.Bacc("TRN2", target_bir_lowering=False, debug=False,
                   enable_asserts=False, num_devices=NCORES)
    p_in = nc.dram_tensor("p_in", [P, FREE], mybir.dt.float32, kind="ExternalInput")
    t_in = nc.dram_tensor("t_in", [P, FREE], mybir.dt.float32, kind="ExternalInput")
    tau_in = nc.dram_tensor("tau_in", [P, 1], mybir.dt.float32, kind="ExternalInput")
    ntau_in = nc.dram_tensor("ntau_in", [P, 1], mybir.dt.float32, kind="ExternalInput")
    out_ra = nc.dram_tensor("out_ra", [P, NCOLS], mybir.dt.float32, kind="ExternalOutput")
    out_lq = nc.dram_tensor("out_lq", [P, NCOLS], mybir.dt.float32, kind="ExternalOutput")
    out_cnt = nc.dram_tensor("out_cnt", [P, NCOLS], mybir.dt.float32, kind="ExternalOutput")

    f32 = mybir.dt.float32
    AF = mybir.ActivationFunctionType
    OP = mybir.AluOpType

    with tile.TileContext(nc) as tc:
        with tc.tile_pool(name="io", bufs=4) as io_pool, \
             tc.tile_pool(name="work", bufs=3) as work, \
             tc.tile_pool(name="junkp", bufs=2) as junkp, \
             tc.tile_pool(name="accs", bufs=1) as accs:
            g_eng = nc.gpsimd if G_ON_GPSIMD else nc.vector
            wdt = f32 if PREC == "f32" else mybir.dt.float16
            do_cast = PREC != "f32" and CAST_T
            t_dt = wdt if do_cast else f32
            cast_eng = nc.gpsimd if do_cast else nc.sync
            sel_flag = {(i, s): d for (i, s, _, d) in plan}

            def pieces_for(i):
                if i == 0:
                    return PIECES_FIRST
                if i == NCH - 1:
                    return PIECES_LAST
                return [(0, CHUNKS[i])]

            # First p piece DMA goes out before anything else so the first
            # LN starts as early as possible.
            ch0 = CHUNKS[0]
            pt0 = io_pool.tile([P, ch0], f32, tag="p")
            tt0 = io_pool.tile([P, ch0], t_dt, tag="t")
            for s, n in pieces_for(0):
                nc.sync.dma_start(pt0[:, s:s + n], p_in.ap()[:, s:s + n])
                cast_eng.dma_start(tt0[:, s:s + n], t_in.ap()[:, s:s + n])

            tau = accs.tile([P, 1], f32)
            ntau = accs.tile([P, 1], f32)
            racc = accs.tile([P, NCOLS], f32)
            lacc = accs.tile([P, NCOLS], f32)
            cacc = accs.tile([P, NCOLS], f32) if COUNT_ON else None
            nc.sync.dma_start(tau[:], tau_in.ap())
            nc.sync.dma_start(ntau[:], ntau_in.ap())

            col = 0
            off = 0
            for i, ch in enumerate(CHUNKS):
                base = off
                off += ch
                if i == 0:
                    pt, tt = pt0, tt0
                else:
                    pt = io_pool.tile([P, ch], f32, tag="p")
                    tt = io_pool.tile([P, ch], t_dt, tag="t")
                    for s, n in pieces_for(i):
                        nc.sync.dma_start(pt[:, s:s + n],
                                          p_in.ap()[:, base + s:base + s + n])
                        # HWDGE can't cast; gpsimd DGE casts f32->f16.
                        cast_eng.dma_start(tt[:, s:s + n],
                                           t_in.ap()[:, base + s:base + s + n])

                lp = work.tile([P, ch], wdt, tag="lp")
                if PREC != "f32":
                    lq = work.tile([P, ch], wdt, tag="lq")
                else:
                    lq = pt
                for s, n in pieces_for(i):
                    sl = slice(s, s + n)
                    nc.scalar.activation(lp[:, sl], pt[:, sl], AF.Ln)
                    # ln(1-p), with a free per-lane sum(lq) via accum_out
                    nc.scalar.activation(lq[:, sl], pt[:, sl], AF.Ln,
                                         bias=1.0, scale=-1.0,
                                         accum_out=lacc[:, col:col + 1])
                    # g = lq - lp  (onto lp)
                    g_eng.tensor_tensor(out=lp[:, sl], in0=lq[:, sl],
                                        in1=lp[:, sl], op=OP.subtract)
                    # f = t * g  (onto tt)
                    nc.vector.tensor_tensor(out=tt[:, sl], in0=tt[:, sl],
                                            in1=lp[:, sl], op=OP.mult)
                    # Fused drain: max(f - tau, lq) = lq + relu(loss - tau),
                    # so accum - sum(lq) gives this piece's relu-sum. One op
                    # replaces loss-subtract + selection.
                    junk2 = junkp.tile([P, ch], wdt, tag="junk2")
                    nc.vector.scalar_tensor_tensor(
                        out=junk2[:, :n], in0=tt[:, sl], scalar=tau[:],
                        in1=lq[:, sl], op0=OP.subtract, op1=OP.max,
                        accum_out=racc[:, col:col + 1])
                    if COUNT_ON:
                        # loss = f - lq (onto lq), then count(loss > tau)
                        nc.vector.tensor_tensor(out=lq[:, sl], in0=tt[:, sl],
                                                in1=lq[:, sl],
                                                op=OP.subtract)
                        junk1 = junkp.tile([P, ch], wdt, tag="junk1")
                        nc.vector.tensor_scalar(
                            out=junk1[:, :n], in0=lq[:, sl], scalar1=tau[:],
                            scalar2=None, op0=OP.is_gt, op1=OP.add,
                            accum_out=cacc[:, col:col + 1])
                    col += 1

            nc.sync.dma_start(out_ra.ap(), racc[:])
            nc.sync.dma_start(out_lq.ap(), lacc[:])
            if COUNT_ON:
                nc.sync.dma_start(out_cnt.ap(), cacc[:])
    nc.compile()
    return nc


def _get_nc():
    global _CACHED_NC
    if _CACHED_NC is None:
        _CACHED_NC = _build_nc()
    return _CACHED_NC


def _pilot(p_flat, t_flat, k):
    """Host pilot on a strided subsample: estimate the k-th largest loss tau
    and the expected A = sum(relu(loss - tau)) for the sanity guard."""
    ps = p_flat[::64].astype(np.float64)
    ts = t_flat[::64].astype(np.float64)
    loss = -(ts * np.clip(np.log(ps), -100.0, None)
             + (1.0 - ts) * np.clip(np.log1p(-ps), -100.0, None))
    n = loss.size
    if k <= 0:
        tau = 0.0
    else:
        kk = min(n - 1, max(1, int(round(n * (k / N_TOTAL)))))
        tau = float(np.partition(loss, n - kk)[n - kk])
    a_pred = float(np.maximum(loss - tau, 0.0).mean()) * N_TOTAL
    return tau, a_pred


def _run_device_pass(nc, p_full, t_full, tau):
    """One full pass: returns (A = sum(relu(loss - tau)), C = count(loss > tau))."""
    global LAST_RESULTS
    in_maps = []
    tau_arr = np.full((P, 1), tau, np.float32)
    ntau_arr = np.full((P, 1), -tau, np.float32)
    per_img = PER_CORE // (B // NCORES)  # elements per image
    imgs_per_core = B // NCORES
    for c in range(NCORES):
        lo = c * imgs_per_core * per_img
        hi = lo + PER_CORE
        in_maps.append({
            "p_in": p_full[lo:hi].reshape(P, FREE),
            "t_in": t_full[lo:hi].reshape(P, FREE),
            "tau_in": tau_arr,
            "ntau_in": ntau_arr,
        })
    res = run_bass_kernel_spmd(nc, in_maps, core_ids=list(range(NCORES)),
                               trace=TRACE)
    LAST_RESULTS = res
    A = 0.0
    C = 0.0
    for c in range(NCORES):
        ra = res.results[c]["out_ra"].astype(np.float64)
        lq = res.results[c]["out_lq"].astype(np.float64)
        A += float(ra.sum()) - float(lq.sum())
        if COUNT_ON:
            C += float(res.results[c]["out_cnt"].astype(np.float64).sum())
    return A, C


def kernel(input, target, it):
    p_full = np.ascontiguousarray(np.asarray(input, dtype=np.float32)).ravel()
    t_full = np.ascontiguousarray(np.asarray(target, dtype=np.float32)).ravel()
    it_val = int(np.asarray(it))
    nc = _get_nc()

    if it_val < START_WARM:
        # Plain mean of all losses: tau=0 makes relu(loss-0)=loss (loss >= 0).
        _, a_pred = _pilot(p_full, t_full, 0)
        A, _ = _run_device_pass(nc, p_full, t_full, 0.0)
        assert abs(A - a_pred) <= 0.2 * abs(a_pred) + 1e-6, (A, a_pred)
        return np.float32(A / N_TOTAL), 1.0

    k = int(N_TOTAL * TOP_P)
    tau, a_pred = _pilot(p_full, t_full, k)
    A, C = _run_device_pass(nc, p_full, t_full, tau)
    # Guard: the device A must agree with the pilot's prediction to ~20%
    # (iid sampling errors are ~0.3%; a gross mismatch means the strided
    # pilot was unrepresentative). Fall back to exact bisection with the
    # count variant of the kernel in that case.
    if abs(A - a_pred) > 0.2 * abs(a_pred) + 1e-6:
        global COUNT_ON, _CACHED_NC
        COUNT_ON, _CACHED_NC = True, None
        nc = _get_nc()
        A, C = _run_device_pass(nc, p_full, t_full, tau)
        lo_t, hi_t = 0.0, 101.0
        for _ in range(40):
            if abs(C - k) <= 0.02 * k:
                break
            if C > k:
                lo_t = tau
            else:
                hi_t = tau
            tau = 0.5 * (lo_t + hi_t)
            A, C = _run_device_pass(nc, p_full, t_full, tau)
    return np.float32(tau + A / k), TOP_P
